# revision 41
# baseline (speedup 1.0000x reference)
"""Trainium2 Bass kernel for masked multi-head attention + depthwise residual conv.

Reference computation (per batch b):
    qkv = x @ W_qkv -> split (3, heads=8, d=64)
    dots = q @ k^T * d**-0.5 ; key-masked softmax
    out  = attn @ v + depthwise_conv33_seq(v)
    out  = out @ W_out + b_out ; row-masked to 0

Sharding: 16 (batch, head-pair) units -> 8 cores, each core handles one batch
and two adjacent heads, producing a partial [2048, 512] projection output.
Host sums the 4 partials per batch, adds b_out, applies the row mask.

Device-side layout: dots are computed transposed, dotsT[j, i] (keys on
partitions), so the key mask folds into the exp bias and softmax needs no
max-pass; the denominator l_i comes from a ones-column in the attn@v lhsT.
The depthwise conv is a banded-Toeplitz matmul with host-precomputed blocks.

All big matmuls run in fp8e4m3 DoubleRow mode (2 contraction rows per PE
pass): qkv contracts kc-chunk pairs, dots contracts d split as 32x2 per head,
attn@v and conv contract key-chunk pairs, and the projection contracts the
head pair. Accumulation stays fp32 in PSUM; exp runs on Act in f32->fp8.
W_qkv/W_out are host-scaled x4 to stay in fp8 normal range (compensated by
the exp scale, the 4/l broadcast, tb/4 taps, and a 0.25 output scale); exp
carries a -2 bias shift (cancels in softmax) to keep pt below fp8e4m3 max.
"""

import sys

sys.path.insert(0, "/opt/trn_rl_repo")

from contextlib import ExitStack

import numpy as np

import concourse.bass as bass
import concourse.tile as tile
from concourse import bacc, mybir

F32 = mybir.dt.float32
BF16 = mybir.dt.bfloat16
FP8 = mybir.dt.float8e4
DR = mybir.MatmulPerfMode.DoubleRow

HEADS = 8
D = 64
DIM = 512
KER = 33
PAD = KER // 2
SCALE = D ** -0.5
B = 2
N = 2048
NCORES = 8
NEG = -1.0e30
WSC = 4.0  # host weight prescale (fp8 range)


def _build_body(ctx: ExitStack, tc: "tile.TileContext", ins, outs, dbg=None):
    nc = tc.nc
    xT, wqk, wv, wout, tblk, mbias = ins
    out = outs[0]

    Exp = mybir.ActivationFunctionType.Exp
    Copy = mybir.ActivationFunctionType.Copy

    singles = ctx.enter_context(tc.tile_pool(name="singles", bufs=1))
    ptpool = ctx.enter_context(tc.tile_pool(name="ptpool", bufs=10))
    lpool = ctx.enter_context(tc.tile_pool(name="lpool", bufs=2))
    opool = ctx.enter_context(tc.tile_pool(name="opool", bufs=2))
    psA = ctx.enter_context(tc.tile_pool(name="psA", bufs=2, space="PSUM"))
    psAV = ctx.enter_context(tc.tile_pool(name="psAV", bufs=2, space="PSUM"))
    psR = ctx.enter_context(tc.tile_pool(name="psR", bufs=2, space="PSUM"))

    # ---- resident SBUF tensors ----
    xt_sb = singles.tile([128, 4, N], BF16)  # xT chunks: [p, kc, i]
    xTr = xT.rearrange("(c p) i -> c p i", p=128)
    for kc in range(4):
        nc.sync.dma_start(out=xt_sb[:, kc, :], in_=xTr[kc])
    wqk_sb = singles.tile([128, 4, 256], BF16)  # [p, kc, fc*128+m*64+(h,dlow)]
    nc.sync.dma_start(out=wqk_sb[:], in_=wqk)
    mb_sb = singles.tile([128, 16], F32)  # mask bias per j: [p, jc]
    nc.sync.dma_start(out=mb_sb[:], in_=mbias[:, :])
    wv_sb = singles.tile([128, 4, 128], BF16)  # [p, kc, (h,d)]
    wout_sb = singles.tile([64, 2, 512], BF16)  # [d, h, f]
    tb_sb = singles.tile([128, 12, DIM], BF16)  # conv blocks: [p, h*6+m, fi]

    ident = singles.tile([128, 128], BF16)
    from concourse.masks import make_identity

    make_identity(nc, ident[:])
    ones_l = singles.tile([65, 64], BF16)  # row 64 used (av is true-scale)
    nc.gpsimd.memset(ones_l[:], 1.0)
    # v in [j, d] layout, head-major, 128-wide pair blocks: [d(64) | 1 | pad]
    v_sbh = []
    for h in range(2):
        t = singles.tile([128, 16, 128], FP8, tag=f"v{h}", name=f"v_sb{h}")
        nc.gpsimd.memset(t[:, :, 64:128], 0.0)
        nc.gpsimd.memset(t[:, :, 64], 1.0)
        v_sbh.append(t)

    v_cv = singles.tile([128, 16, 2, 64], BF16)  # conv v: [j, jc, h, d]
    qt_sb = singles.tile([128, N], BF16)  # qT: rows h*64+d, cols i
    kt_sb = singles.tile([128, N], BF16)
    vt_sb = singles.tile([128, N], BF16)  # vT: rows h*64+d, cols i
    projin = singles.tile([64, 16, 2, 128], BF16)  # [d, iblk, h, i]

    # ---- q/k/v projections (bf16); q/k quantized once to fp8 on the cast ----
    def emit_qk(fc, ic):
        ps = psA.tile([128, 512], F32, tag="A")
        for kc in range(4):
            nc.tensor.matmul(
                ps[:],
                wqk_sb[:, kc, fc * 128 : (fc + 1) * 128],
                xt_sb[:, kc, ic * 512 : (ic + 1) * 512],
                start=(kc == 0),
                stop=(kc == 3),
            )
        dst = qt_sb if fc == 0 else kt_sb
        if fc == 0:
            nc.scalar.activation(dst[:, ic * 512 : (ic + 1) * 512], ps[:], Copy)
        else:
            nc.vector.tensor_copy(dst[:, ic * 512 : (ic + 1) * 512], ps[:])

    def emit_v(ic):
        ps = psA.tile([128, 512], F32, tag="A")
        for kc in range(4):
            nc.tensor.matmul(
                ps[:],
                wv_sb[:, kc, :],
                xt_sb[:, kc, ic * 512 : (ic + 1) * 512],
                start=(kc == 0),
                stop=(kc == 3),
            )
        nc.vector.tensor_copy(vt_sb[:, ic * 512 : (ic + 1) * 512], ps[:])

    def emit_transpose(jc):
        # v natural [j, d]: PE-transpose of vT 128x128 blocks (psAV tag: all
        # transposes complete before the first attn@v accumulator allocates)
        ps = psAV.tile([128, 128], BF16, tag="AV")
        nc.tensor.transpose(ps[:], vt_sb[:, jc * 128 : (jc + 1) * 128], ident[:])
        nc.vector.tensor_copy(v_cv[:, jc, :, :], ps[:])
        for h in range(2):
            nc.gpsimd.tensor_copy(v_sbh[h][:, jc, 0:64], v_cv[:, jc, h, :])

    if dbg is not None:
        nc.sync.dma_start(out=dbg["qt"], in_=qt_sb[:])
        nc.sync.dma_start(out=dbg["kt"], in_=kt_sb[:])
        nc.sync.dma_start(out=dbg["v0"], in_=v_sbh[0][:])
        nc.sync.dma_start(out=dbg["v1"], in_=v_sbh[1][:])

    # ---- attention: dots (DR over d pairs) -> exp -> attn@v (DR over jc pairs)
    def emit_pair(ic, p):
        """dots + exp for key chunks (2p, 2p+1) of chunk ic; returns pt2."""
        pt2 = ptpool.tile([128, 2, 2, 512], FP8, tag="pt")  # [j, h, member, i]
        i5 = slice(ic * 512, (ic + 1) * 512)
        for mem in range(2):
            jc = 2 * p + mem
            dots = psA.tile([128, 2, 512], F32, tag="A")
            for h in range(2):
                nc.tensor.matmul(
                    dots[:, h, :],
                    kt_sb[h * 64 : (h + 1) * 64, jc * 128 : (jc + 1) * 128],
                    qt_sb[h * 64 : (h + 1) * 64, i5],
                    start=True,
                    stop=True,
                )
            nc.scalar.activation(
                pt2[:, :, mem, :],
                dots[:],
                Exp,
                bias=mb_sb[:, jc : jc + 1],
                scale=SCALE,
            )
        return pt2

    def emit_conv(ic):
        """Banded-Toeplitz conv for chunk ic -> [h0, h1] psum tiles."""
        res = []
        for h in range(2):
            rps = psR.tile([64, 4, 128], F32, tag="R")
            ms = [m for m in range(6) if 0 <= ic * 512 - 128 + m * 128 < N]
            for mi, m in enumerate(ms):
                jcm = 4 * ic + m - 1
                nc.tensor.matmul(
                    rps[:],
                    v_cv[:, jcm, h, :],
                    tb_sb[:, h * 6 + m, :],
                    start=(mi == 0),
                    stop=(mi == len(ms) - 1),
                )
            res.append(rps)
        return res

    def emit_proj(ic):
        """Head-pair projection of chunk ic + output DMA."""
        osb = opool.tile([128, 4, DIM], F32, tag="osb")
        for sub in range(4):
            pp = psA.tile([128, 512], F32, tag="A")
            for h in range(2):
                nc.tensor.matmul(
                    pp[:],
                    projin[:, 4 * ic + sub, h, :],
                    wout_sb[:, h, :],
                    start=(h == 0),
                    stop=(h == 1),
                )
            nc.vector.tensor_copy(osb[:, sub, :], pp[:])
        nc.sync.dma_start(
            out=out.rearrange("(ic c p) f -> ic p c f", c=4, p=128)[ic],
            in_=osb[:],
        )

    # software pipeline: dots+exp run 4 pairs ahead of attn@v so Act never
    # starves through the chunk-boundary epilogue; conv for the next chunk
    # fills the PE under the DVE epilogue; the projection of chunk ic drains
    # early in chunk ic+1's pair loop. Startup emits chunk-0 k/q first so the
    # exp pipeline starts while the rest of qkv still runs on the PE.
    emit_qk(1, 0)
    emit_qk(0, 0)
    pend = {(0, 0): emit_pair(0, 0), (0, 1): emit_pair(0, 1)}
    nc.sync.dma_start(out=wv_sb[:], in_=wv)
    nc.sync.dma_start(out=tb_sb[:], in_=tblk.rearrange("g p f -> p g f"))
    nc.sync.dma_start(out=wout_sb[:], in_=wout)
    emit_qk(1, 1)
    pend[(0, 2)] = emit_pair(0, 2)
    pend[(0, 3)] = emit_pair(0, 3)
    for kic in (2, 3):
        emit_qk(1, kic)
    for p in (4, 5, 6, 7):
        pend[(0, p)] = emit_pair(0, p)
    for qic in (1, 2, 3):
        emit_qk(0, qic)
    for vic in range(4):
        emit_v(vic)
    for jc in range(16):
        emit_transpose(jc)
    res = emit_conv(0)
    proj_todo = None
    for ic in range(4):
        avh = []
        for h in range(2):
            avh.append(psAV.tile([128, 4, 128], F32, tag="AV", name="av"))

        for p in range(8):
            nic, np_ = (ic, p + 4) if p + 4 < 8 else (ic + 1, p - 4)
            if nic < 4 and (nic, np_) not in pend:
                pend[(nic, np_)] = emit_pair(nic, np_)
            pt2 = pend.pop((ic, p))
            for h in range(2):
                nc.tensor.matmul(
                    avh[h][:],
                    v_sbh[h][:, 2 * p : 2 * p + 2, :],
                    pt2[:, h, :, :],
                    start=(p == 0),
                    stop=(p == 7),
                    perf_mode=DR,
                )
            if p == 2 and proj_todo is not None:
                emit_proj(proj_todo)
                proj_todo = None

        # epilogue: l (row 64 of av) -> l broadcast -> rc=1/l -> projin
        l_sb = lpool.tile([65, 2, 4, 128], BF16, tag="l")
        nc.vector.tensor_copy(l_sb[64:65, 0, :, :], avh[0][64:65, :, :])
        nc.scalar.activation(l_sb[64:65, 1, :, :], avh[1][64:65, :, :], Copy)
        bc = psA.tile([64, 2, 4, 128], F32, tag="A")
        for h in range(2):
            nc.tensor.matmul(
                bc[:, h, :, :],
                ones_l[64:65, 0:64],
                l_sb[64:65, h, :, :],
                start=True,
                stop=True,
            )
        rc = lpool.tile([64, 2, 4, 128], F32, tag="rc")
        for h in range(2):
            nc.vector.reciprocal_approx_fast(rc[:, h, :, :], bc[:, h, :, :])
        for h in range(2):
            nc.vector.tensor_mul(
                projin[:, 4 * ic : 4 * ic + 4, h, :],
                avh[h][0:64, :, :],
                rc[:, h, :, :],
            )
            nc.vector.tensor_add(
                projin[:, 4 * ic : 4 * ic + 4, h, :],
                projin[:, 4 * ic : 4 * ic + 4, h, :],
                res[h][:],
            )
        if dbg is not None and ic == 0:
            asb = singles.tile([128, 4, 128], F32, tag="dbgav", name="dbgav")
            nc.vector.tensor_copy(asb[:], avh[0][:])
            nc.sync.dma_start(out=dbg["av0"], in_=asb[:])
        if ic < 3:
            res = emit_conv(ic + 1)
            proj_todo = ic
        else:
            emit_proj(ic)

    if dbg is not None:
        nc.sync.dma_start(out=dbg["projin"], in_=projin[:])


_NC_CACHE = {}


def _dram_tensors(nc):
    ins = [
        nc.dram_tensor("xT", [DIM, N], BF16, kind="ExternalInput").ap(),
        nc.dram_tensor("wqk", [128, 4, 256], BF16, kind="ExternalInput").ap(),
        nc.dram_tensor("wv", [128, 4, 128], BF16, kind="ExternalInput").ap(),
        nc.dram_tensor("wout", [64, 2, 512], BF16, kind="ExternalInput").ap(),
        nc.dram_tensor("tblk", [12, 128, DIM], BF16, kind="ExternalInput").ap(),
        nc.dram_tensor("mbias", [128, 16], F32, kind="ExternalInput").ap(),
    ]
    outs = [nc.dram_tensor("out", [N, DIM], F32, kind="ExternalOutput").ap()]
    return ins, outs


def _get_nc(reps: int = 1):
    if reps in _NC_CACHE:
        return _NC_CACHE[reps]
    nc = bacc.Bacc(
        "TRN2",
        target_bir_lowering=False,
        debug=False,
        num_devices=NCORES,
    )
    ins, outs = _dram_tensors(nc)
    with tile.TileContext(nc) as tc:
        if reps == 1:
            with ExitStack() as ctx:
                _build_body(ctx, tc, ins, outs)
        else:
            with tc.For_i(0, reps, 1):
                with ExitStack() as ctx:
                    _build_body(ctx, tc, ins, outs)
    nc.compile()
    _NC_CACHE[reps] = nc
    return nc


def _get_nc_debug():
    nc = bacc.Bacc(
        "TRN2", target_bir_lowering=False, debug=False, num_devices=NCORES
    )
    ins, outs = _dram_tensors(nc)
    dbg = {
        "qt": nc.dram_tensor("d_qt", [64, 4, 2, 512], FP8, kind="ExternalOutput").ap(),
        "kt": nc.dram_tensor("d_kt", [64, 16, 2, 128], FP8, kind="ExternalOutput").ap(),
        "v0": nc.dram_tensor("d_v0", [128, 16, 128], FP8, kind="ExternalOutput").ap(),
        "v1": nc.dram_tensor("d_v1", [128, 16, 128], FP8, kind="ExternalOutput").ap(),
        "av0": nc.dram_tensor("d_av0", [128, 4, 128], F32, kind="ExternalOutput").ap(),
        "projin": nc.dram_tensor(
            "d_projin", [64, 16, 2, 128], BF16, kind="ExternalOutput"
        ).ap(),
    }
    with tile.TileContext(nc) as tc:
        with ExitStack() as ctx:
            _build_body(ctx, tc, ins, outs, dbg=dbg)
    nc.compile()
    return nc


def _conv_blocks(conv_w_pair: np.ndarray) -> np.ndarray:
    """[2, 33] taps -> [12, 128, 512] banded T^T blocks (scaled 1/WSC).

    Block (h, m) holds T^T[j0+pj, i0+fi] = w_h[(j0-i0) + pj - fi + PAD] with
    j0-i0 = -128 + 128*m, zero outside the +-PAD band.
    """
    blocks = np.zeros((2, 6, 128, DIM), np.float32)
    pj = np.arange(128)[:, None]
    fi = np.arange(DIM)[None, :]
    for h in range(2):
        w = conv_w_pair[h]
        for m in range(6):
            idx = (-128 + 128 * m) + pj - fi + PAD
            valid = (idx >= 0) & (idx < KER)
            blocks[h, m][valid] = w[idx[valid]]
    return blocks.reshape(12, 128, DIM)


def _make_in_maps(x, mask, W_qkv, W_out, conv_w):
    import ml_dtypes

    f8 = ml_dtypes.float8_e4m3
    x = np.asarray(x, np.float32)
    mask = np.asarray(mask)
    W_qkv = np.asarray(W_qkv, np.float32)
    W_out = np.asarray(W_out, np.float32)
    conv_w = np.asarray(conv_w, np.float32)

    bf = ml_dtypes.bfloat16
    c64 = np.arange(64)
    c128 = np.arange(128)
    in_maps = []
    for core in range(NCORES):
        b = core // 4
        h0 = (core % 4) * 2
        # wqk: [p, kc, fc*128+c] with c=(h*64+d)
        wqk_cols = np.zeros(256, np.int64)
        for fc in range(2):
            wqk_cols[fc * 128 : fc * 128 + 128] = (
                fc * DIM + (h0 + c128 // 64) * 64 + c128 % 64
            )
        wqk_b = W_qkv[:, wqk_cols].reshape(4, 128, 256).transpose(1, 0, 2)
        # wv: [p, kc, c] with c=(h*64+d)
        colsv = 2 * DIM + (h0 + c128 // 64) * 64 + c128 % 64
        wv_b = W_qkv[:, colsv].reshape(4, 128, 128).transpose(1, 0, 2)
        # wout: [d, h, f]
        wout_b = np.stack(
            [W_out[(h0 + h) * 64 : (h0 + h + 1) * 64, :] for h in range(2)],
            axis=1,
        )
        mb = np.where(mask[b], -2.0, NEG).astype(np.float32)
        in_maps.append(
            {
                "xT": np.ascontiguousarray(x[b].T).astype(bf),
                "wqk": np.ascontiguousarray(wqk_b).astype(bf),
                "wv": np.ascontiguousarray(wv_b).astype(bf),
                "wout": wout_b.astype(bf),
                "tblk": _conv_blocks(conv_w[h0 : h0 + 2, 0, :, 0]).astype(bf),
                "mbias": np.ascontiguousarray(mb.reshape(16, 128).T),
            }
        )

    return in_maps


def _combine(results, mask, b_out):
    out = np.zeros((B, N, DIM), np.float32)
    for core in range(NCORES):
        out[core // 4] += np.asarray(results[core]["out"], np.float32)
    out += np.asarray(b_out, np.float32)[None, None, :]
    out *= np.asarray(mask)[:, :, None].astype(np.float32)
    return out


def kernel(x, mask, W_qkv, W_out, b_out, conv_w):
    from concourse.bass_utils import run_bass_kernel_spmd

    nc = _get_nc()
    in_maps = _make_in_maps(x, mask, W_qkv, W_out, conv_w)
    results = run_bass_kernel_spmd(nc, in_maps, list(range(NCORES))).results
    return _combine(results, mask, b_out)


# revision 42
# speedup vs baseline: 1.0236x; 1.0236x over previous
"""Trainium2 Bass kernel for masked multi-head attention + depthwise residual conv.

Reference computation (per batch b):
    qkv = x @ W_qkv -> split (3, heads=8, d=64)
    dots = q @ k^T * d**-0.5 ; key-masked softmax
    out  = attn @ v + depthwise_conv33_seq(v)
    out  = out @ W_out + b_out ; row-masked to 0

Sharding: 16 (batch, head-pair) units -> 8 cores, each core handles one batch
and two adjacent heads, producing a partial [2048, 512] projection output.
Host sums the 4 partials per batch, adds b_out, applies the row mask.

Device-side layout: dots are computed transposed, dotsT[j, i] (keys on
partitions), so the key mask folds into the exp bias and softmax needs no
max-pass; the denominator l_i comes from a ones-column in the attn@v lhsT.
The depthwise conv is a banded-Toeplitz matmul with host-precomputed blocks.

qkv/dots/conv/projection matmuls run in bf16 (1 PE cycle per output row);
attn@v runs in fp8e4m3 DoubleRow, contracting key-chunk pairs in one stream.
q/k are quantized once to fp8 on the PSUM cast; exp writes fp8 attention
numerators with a -2 bias shift (cancels in softmax) so they fit fp8e4m3.
Accumulation stays fp32 in PSUM. The emission order software-pipelines
dots+exp four key-pairs ahead of attn@v so the Act engine (the exp
bottleneck) stays fed across chunk-boundary epilogues.
"""

import sys

sys.path.insert(0, "/opt/trn_rl_repo")

from contextlib import ExitStack

import numpy as np

import concourse.bass as bass
import concourse.tile as tile
from concourse import bacc, mybir

F32 = mybir.dt.float32
BF16 = mybir.dt.bfloat16
FP8 = mybir.dt.float8e4
DR = mybir.MatmulPerfMode.DoubleRow

HEADS = 8
D = 64
DIM = 512
KER = 33
PAD = KER // 2
SCALE = D ** -0.5
B = 2
N = 2048
NCORES = 8
NEG = -1.0e30
WSC = 4.0  # host weight prescale (fp8 range)


def _build_body(ctx: ExitStack, tc: "tile.TileContext", ins, outs, dbg=None):
    nc = tc.nc
    xT, wqk, wv, wout, tblk, mbias = ins
    out = outs[0]

    Exp = mybir.ActivationFunctionType.Exp
    Copy = mybir.ActivationFunctionType.Copy

    singles = ctx.enter_context(tc.tile_pool(name="singles", bufs=1))
    ptpool = ctx.enter_context(tc.tile_pool(name="ptpool", bufs=10))
    lpool = ctx.enter_context(tc.tile_pool(name="lpool", bufs=2))
    opool = ctx.enter_context(tc.tile_pool(name="opool", bufs=2))
    psA = ctx.enter_context(tc.tile_pool(name="psA", bufs=2, space="PSUM"))
    psAV = ctx.enter_context(tc.tile_pool(name="psAV", bufs=2, space="PSUM"))
    psR = ctx.enter_context(tc.tile_pool(name="psR", bufs=2, space="PSUM"))

    # ---- resident SBUF tensors ----
    xt_sb = singles.tile([128, 4, N], BF16)  # xT chunks: [p, kc, i]
    xTr = xT.rearrange("(c p) i -> c p i", p=128)
    for kc in range(4):
        nc.sync.dma_start(out=xt_sb[:, kc, :], in_=xTr[kc])
    wqk_sb = singles.tile([128, 4, 256], BF16)  # [p, kc, fc*128+m*64+(h,dlow)]
    nc.sync.dma_start(out=wqk_sb[:], in_=wqk)
    mb_sb = singles.tile([128, 16], F32)  # mask bias per j: [p, jc]
    nc.sync.dma_start(out=mb_sb[:], in_=mbias[:, :])
    wv_sb = singles.tile([128, 4, 128], BF16)  # [p, kc, (h,d)]
    wout_sb = singles.tile([64, 2, 512], BF16)  # [d, h, f]
    tb_sb = singles.tile([128, 12, DIM], BF16)  # conv blocks: [p, h*6+m, fi]

    ident = singles.tile([128, 128], BF16)
    from concourse.masks import make_identity

    make_identity(nc, ident[:])
    ones_l = singles.tile([65, 64], BF16)  # row 64 used (av is true-scale)
    nc.gpsimd.memset(ones_l[:], 1.0)
    # v in [j, d] layout, head-major, 128-wide pair blocks: [d(64) | 1 | pad]
    v_sbh = []
    for h in range(2):
        t = singles.tile([128, 16, 128], FP8, tag=f"v{h}", name=f"v_sb{h}")
        nc.gpsimd.memset(t[:, :, 64:128], 0.0)
        nc.gpsimd.memset(t[:, :, 64], 1.0)
        v_sbh.append(t)

    v_cv = singles.tile([128, 16, 2, 64], BF16)  # conv v: [j, jc, h, d]
    qt_sb = singles.tile([128, N], BF16)  # qT: rows h*64+d, cols i
    kt_sb = singles.tile([128, N], BF16)
    vt_sb = singles.tile([128, N], BF16)  # vT: rows h*64+d, cols i
    projin = singles.tile([64, 16, 2, 128], BF16)  # [d, iblk, h, i]

    # ---- q/k/v projections (bf16); q/k quantized once to fp8 on the cast ----
    def emit_qk(fc, ic):
        ps = psA.tile([128, 512], F32, tag="A")
        for kc in range(4):
            nc.tensor.matmul(
                ps[:],
                wqk_sb[:, kc, fc * 128 : (fc + 1) * 128],
                xt_sb[:, kc, ic * 512 : (ic + 1) * 512],
                start=(kc == 0),
                stop=(kc == 3),
            )
        dst = qt_sb if fc == 0 else kt_sb
        if fc == 0:
            nc.scalar.activation(dst[:, ic * 512 : (ic + 1) * 512], ps[:], Copy)
        else:
            nc.vector.tensor_copy(dst[:, ic * 512 : (ic + 1) * 512], ps[:])

    def emit_v(ic):
        ps = psA.tile([128, 512], F32, tag="A")
        for kc in range(4):
            nc.tensor.matmul(
                ps[:],
                wv_sb[:, kc, :],
                xt_sb[:, kc, ic * 512 : (ic + 1) * 512],
                start=(kc == 0),
                stop=(kc == 3),
            )
        nc.vector.tensor_copy(vt_sb[:, ic * 512 : (ic + 1) * 512], ps[:])

    def emit_transpose(jc):
        # v natural [j, d]: PE-transpose of vT 128x128 blocks (psAV tag: all
        # transposes complete before the first attn@v accumulator allocates)
        ps = psAV.tile([128, 128], BF16, tag="AV")
        nc.tensor.transpose(ps[:], vt_sb[:, jc * 128 : (jc + 1) * 128], ident[:])
        nc.vector.tensor_copy(v_cv[:, jc, :, :], ps[:])
        for h in range(2):
            nc.gpsimd.tensor_copy(v_sbh[h][:, jc, 0:64], v_cv[:, jc, h, :])

    if dbg is not None:
        nc.sync.dma_start(out=dbg["qt"], in_=qt_sb[:])
        nc.sync.dma_start(out=dbg["kt"], in_=kt_sb[:])
        nc.sync.dma_start(out=dbg["v0"], in_=v_sbh[0][:])
        nc.sync.dma_start(out=dbg["v1"], in_=v_sbh[1][:])

    # ---- attention: dots (DR over d pairs) -> exp -> attn@v (DR over jc pairs)
    def emit_pair(ic, p):
        """dots + exp for key chunks (2p, 2p+1) of chunk ic; returns pt2."""
        pt2 = ptpool.tile([128, 2, 2, 512], FP8, tag="pt")  # [j, h, member, i]
        i5 = slice(ic * 512, (ic + 1) * 512)
        for mem in range(2):
            jc = 2 * p + mem
            dots = psA.tile([128, 2, 512], F32, tag="A")
            for h in range(2):
                nc.tensor.matmul(
                    dots[:, h, :],
                    kt_sb[h * 64 : (h + 1) * 64, jc * 128 : (jc + 1) * 128],
                    qt_sb[h * 64 : (h + 1) * 64, i5],
                    start=True,
                    stop=True,
                )
            nc.scalar.activation(
                pt2[:, :, mem, :],
                dots[:],
                Exp,
                bias=mb_sb[:, jc : jc + 1],
                scale=SCALE,
            )
        return pt2

    def emit_conv(ic):
        """Banded-Toeplitz conv for chunk ic -> [h0, h1] psum tiles."""
        res = []
        for h in range(2):
            rps = psR.tile([64, 4, 128], F32, tag="R")
            ms = [m for m in range(6) if 0 <= ic * 512 - 128 + m * 128 < N]
            for mi, m in enumerate(ms):
                jcm = 4 * ic + m - 1
                nc.tensor.matmul(
                    rps[:],
                    v_cv[:, jcm, h, :],
                    tb_sb[:, h * 6 + m, :],
                    start=(mi == 0),
                    stop=(mi == len(ms) - 1),
                )
            res.append(rps)
        return res

    def emit_proj(ic):
        """Head-pair projection of chunk ic + output DMA."""
        osb = opool.tile([128, 4, DIM], F32, tag="osb")
        for sub in range(4):
            pp = psA.tile([128, 512], F32, tag="A")
            for h in range(2):
                nc.tensor.matmul(
                    pp[:],
                    projin[:, 4 * ic + sub, h, :],
                    wout_sb[:, h, :],
                    start=(h == 0),
                    stop=(h == 1),
                )
            nc.vector.tensor_copy(osb[:, sub, :], pp[:])
        nc.sync.dma_start(
            out=out.rearrange("(ic c p) f -> ic p c f", c=4, p=128)[ic],
            in_=osb[:],
        )

    # software pipeline: dots+exp run 4 pairs ahead of attn@v so Act never
    # starves through the chunk-boundary epilogue; conv for the next chunk
    # fills the PE under the DVE epilogue; the projection of chunk ic drains
    # early in chunk ic+1's pair loop. Startup emits chunk-0 k/q first so the
    # exp pipeline starts while the rest of qkv still runs on the PE.
    emit_qk(1, 0)
    emit_qk(0, 0)
    pend = {(0, 0): emit_pair(0, 0), (0, 1): emit_pair(0, 1)}
    nc.sync.dma_start(out=wv_sb[:], in_=wv)
    nc.sync.dma_start(out=tb_sb[:], in_=tblk.rearrange("g p f -> p g f"))
    nc.sync.dma_start(out=wout_sb[:], in_=wout)
    emit_qk(1, 1)
    pend[(0, 2)] = emit_pair(0, 2)
    pend[(0, 3)] = emit_pair(0, 3)
    for kic in (2, 3):
        emit_qk(1, kic)
    for qic in (1, 2, 3):
        emit_qk(0, qic)
    for vic in range(4):
        emit_v(vic)
    for jc in range(16):
        emit_transpose(jc)
    res = emit_conv(0)
    proj_todo = None
    for ic in range(4):
        avh = []
        for h in range(2):
            avh.append(psAV.tile([128, 4, 128], F32, tag="AV", name="av"))

        for p in range(8):
            nic, np_ = (ic, p + 4) if p + 4 < 8 else (ic + 1, p - 4)
            if nic < 4 and (nic, np_) not in pend:
                pend[(nic, np_)] = emit_pair(nic, np_)
            pt2 = pend.pop((ic, p))
            for h in range(2):
                nc.tensor.matmul(
                    avh[h][:],
                    v_sbh[h][:, 2 * p : 2 * p + 2, :],
                    pt2[:, h, :, :],
                    start=(p == 0),
                    stop=(p == 7),
                    perf_mode=DR,
                )
            if p == 2 and proj_todo is not None:
                emit_proj(proj_todo)
                proj_todo = None

        # epilogue: l (row 64 of av) -> l broadcast -> rc=1/l -> projin
        l_sb = lpool.tile([65, 2, 4, 128], BF16, tag="l")
        nc.vector.tensor_copy(l_sb[64:65, 0, :, :], avh[0][64:65, :, :])
        nc.scalar.activation(l_sb[64:65, 1, :, :], avh[1][64:65, :, :], Copy)
        bc = psA.tile([64, 2, 4, 128], F32, tag="A")
        for h in range(2):
            nc.tensor.matmul(
                bc[:, h, :, :],
                ones_l[64:65, 0:64],
                l_sb[64:65, h, :, :],
                start=True,
                stop=True,
            )
        rc = lpool.tile([64, 2, 4, 128], F32, tag="rc")
        for h in range(2):
            nc.vector.reciprocal_approx_fast(rc[:, h, :, :], bc[:, h, :, :])
        for h in range(2):
            nc.vector.tensor_mul(
                projin[:, 4 * ic : 4 * ic + 4, h, :],
                avh[h][0:64, :, :],
                rc[:, h, :, :],
            )
            nc.vector.tensor_add(
                projin[:, 4 * ic : 4 * ic + 4, h, :],
                projin[:, 4 * ic : 4 * ic + 4, h, :],
                res[h][:],
            )
        if dbg is not None and ic == 0:
            asb = singles.tile([128, 4, 128], F32, tag="dbgav", name="dbgav")
            nc.vector.tensor_copy(asb[:], avh[0][:])
            nc.sync.dma_start(out=dbg["av0"], in_=asb[:])
        if ic < 3:
            res = emit_conv(ic + 1)
            proj_todo = ic
        else:
            emit_proj(ic)

    if dbg is not None:
        nc.sync.dma_start(out=dbg["projin"], in_=projin[:])


_NC_CACHE = {}


def _dram_tensors(nc):
    ins = [
        nc.dram_tensor("xT", [DIM, N], BF16, kind="ExternalInput").ap(),
        nc.dram_tensor("wqk", [128, 4, 256], BF16, kind="ExternalInput").ap(),
        nc.dram_tensor("wv", [128, 4, 128], BF16, kind="ExternalInput").ap(),
        nc.dram_tensor("wout", [64, 2, 512], BF16, kind="ExternalInput").ap(),
        nc.dram_tensor("tblk", [12, 128, DIM], BF16, kind="ExternalInput").ap(),
        nc.dram_tensor("mbias", [128, 16], F32, kind="ExternalInput").ap(),
    ]
    outs = [nc.dram_tensor("out", [N, DIM], F32, kind="ExternalOutput").ap()]
    return ins, outs


def _get_nc(reps: int = 1):
    if reps in _NC_CACHE:
        return _NC_CACHE[reps]
    nc = bacc.Bacc(
        "TRN2",
        target_bir_lowering=False,
        debug=False,
        num_devices=NCORES,
    )
    ins, outs = _dram_tensors(nc)
    with tile.TileContext(nc) as tc:
        if reps == 1:
            with ExitStack() as ctx:
                _build_body(ctx, tc, ins, outs)
        else:
            with tc.For_i(0, reps, 1):
                with ExitStack() as ctx:
                    _build_body(ctx, tc, ins, outs)
    nc.compile()
    _NC_CACHE[reps] = nc
    return nc


def _get_nc_debug():
    nc = bacc.Bacc(
        "TRN2", target_bir_lowering=False, debug=False, num_devices=NCORES
    )
    ins, outs = _dram_tensors(nc)
    dbg = {
        "qt": nc.dram_tensor("d_qt", [64, 4, 2, 512], FP8, kind="ExternalOutput").ap(),
        "kt": nc.dram_tensor("d_kt", [64, 16, 2, 128], FP8, kind="ExternalOutput").ap(),
        "v0": nc.dram_tensor("d_v0", [128, 16, 128], FP8, kind="ExternalOutput").ap(),
        "v1": nc.dram_tensor("d_v1", [128, 16, 128], FP8, kind="ExternalOutput").ap(),
        "av0": nc.dram_tensor("d_av0", [128, 4, 128], F32, kind="ExternalOutput").ap(),
        "projin": nc.dram_tensor(
            "d_projin", [64, 16, 2, 128], BF16, kind="ExternalOutput"
        ).ap(),
    }
    with tile.TileContext(nc) as tc:
        with ExitStack() as ctx:
            _build_body(ctx, tc, ins, outs, dbg=dbg)
    nc.compile()
    return nc


def _conv_blocks(conv_w_pair: np.ndarray) -> np.ndarray:
    """[2, 33] taps -> [12, 128, 512] banded T^T blocks (scaled 1/WSC).

    Block (h, m) holds T^T[j0+pj, i0+fi] = w_h[(j0-i0) + pj - fi + PAD] with
    j0-i0 = -128 + 128*m, zero outside the +-PAD band.
    """
    blocks = np.zeros((2, 6, 128, DIM), np.float32)
    pj = np.arange(128)[:, None]
    fi = np.arange(DIM)[None, :]
    for h in range(2):
        w = conv_w_pair[h]
        for m in range(6):
            idx = (-128 + 128 * m) + pj - fi + PAD
            valid = (idx >= 0) & (idx < KER)
            blocks[h, m][valid] = w[idx[valid]]
    return blocks.reshape(12, 128, DIM)


def _make_in_maps(x, mask, W_qkv, W_out, conv_w):
    import ml_dtypes

    f8 = ml_dtypes.float8_e4m3
    x = np.asarray(x, np.float32)
    mask = np.asarray(mask)
    W_qkv = np.asarray(W_qkv, np.float32)
    W_out = np.asarray(W_out, np.float32)
    conv_w = np.asarray(conv_w, np.float32)

    bf = ml_dtypes.bfloat16
    c64 = np.arange(64)
    c128 = np.arange(128)
    in_maps = []
    for core in range(NCORES):
        b = core // 4
        h0 = (core % 4) * 2
        # wqk: [p, kc, fc*128+c] with c=(h*64+d)
        wqk_cols = np.zeros(256, np.int64)
        for fc in range(2):
            wqk_cols[fc * 128 : fc * 128 + 128] = (
                fc * DIM + (h0 + c128 // 64) * 64 + c128 % 64
            )
        wqk_b = W_qkv[:, wqk_cols].reshape(4, 128, 256).transpose(1, 0, 2)
        # wv: [p, kc, c] with c=(h*64+d)
        colsv = 2 * DIM + (h0 + c128 // 64) * 64 + c128 % 64
        wv_b = W_qkv[:, colsv].reshape(4, 128, 128).transpose(1, 0, 2)
        # wout: [d, h, f]
        wout_b = np.stack(
            [W_out[(h0 + h) * 64 : (h0 + h + 1) * 64, :] for h in range(2)],
            axis=1,
        )
        mb = np.where(mask[b], -2.0, NEG).astype(np.float32)
        in_maps.append(
            {
                "xT": np.ascontiguousarray(x[b].T).astype(bf),
                "wqk": np.ascontiguousarray(wqk_b).astype(bf),
                "wv": np.ascontiguousarray(wv_b).astype(bf),
                "wout": wout_b.astype(bf),
                "tblk": _conv_blocks(conv_w[h0 : h0 + 2, 0, :, 0]).astype(bf),
                "mbias": np.ascontiguousarray(mb.reshape(16, 128).T),
            }
        )

    return in_maps


def _combine(results, mask, b_out):
    out = np.zeros((B, N, DIM), np.float32)
    for core in range(NCORES):
        out[core // 4] += np.asarray(results[core]["out"], np.float32)
    out += np.asarray(b_out, np.float32)[None, None, :]
    out *= np.asarray(mask)[:, :, None].astype(np.float32)
    return out


def kernel(x, mask, W_qkv, W_out, b_out, conv_w):
    from concourse.bass_utils import run_bass_kernel_spmd

    nc = _get_nc()
    in_maps = _make_in_maps(x, mask, W_qkv, W_out, conv_w)
    results = run_bass_kernel_spmd(nc, in_maps, list(range(NCORES))).results
    return _combine(results, mask, b_out)
